# revision 15
# baseline (speedup 1.0000x reference)
"""Trainium2 Bass kernel: multi-adapter LoRA linear (y = x @ W.T + bias + 2*(x@A_g.T)@B_g.T).

Sharding: data-parallel over the batch dim. Each of the 8 cores gets one batch
element (x.T pre-transposed on host, fp16), the full W.T (fp16, replicated) and
its adapter group's A.T / [2*B.T; bias] (fp16). Matmuls run fp16 with fp32 PSUM
accumulation; bias and the rank-16 LoRA delta are folded into the same PSUM
accumulation group via one extra K=17 matmul per output tile.
"""
import sys

if "/opt/trn_rl_repo" not in sys.path:
    sys.path.insert(0, "/opt/trn_rl_repo")

import numpy as np

B, S, I, O, G, R = 8, 2048, 4096, 4096, 4, 16
OB = 256  # output free-dim tile (fits one PSUM bank of fp32)
SBK = 512  # h-phase moving block

_CACHE = {}


def build(s=S, i=I, o=O, r=R, repeat=1):
    """Build + bacc-compile the per-core program (same SPMD program on all cores).

    repeat>1 emits the whole body N times in one program (timing aid: device
    exec scales with N while host<->device transfer stays constant).
    """
    import concourse.bacc as bacc
    import concourse.mybir as mybir
    import concourse.tile as tile

    f16, f32 = mybir.dt.float16, mybir.dt.float32
    kt = i // 128  # contraction chunks
    mt_n = s // 128  # PSUM partition tiles
    nob = o // OB  # output free-dim chunks
    sbk = min(SBK, s)  # h-phase moving block
    nsb = s // sbk  # h-phase blocks

    nc = bacc.Bacc("TRN2", target_bir_lowering=False, debug=False)
    xT = nc.dram_tensor("xT", [i, s], f16, kind="ExternalInput").ap()
    WT = nc.dram_tensor("WT", [i, o], f16, kind="ExternalInput").ap()
    AT = nc.dram_tensor("AT", [i, r], f16, kind="ExternalInput").ap()
    Baug = nc.dram_tensor("Baug", [r + 1, o], f16, kind="ExternalInput").ap()
    out = nc.dram_tensor("out", [s, o], f32, kind="ExternalOutput").ap()

    with tile.TileContext(nc) as tc:
        with (
            tc.tile_pool(name="xp", bufs=1) as xp,
            tc.tile_pool(name="wp", bufs=2) as wp,
            tc.tile_pool(name="cp", bufs=1) as cp,
            tc.tile_pool(name="op", bufs=4) as op,
            tc.tile_pool(name="pp", bufs=6, space="PSUM") as pp,
            tc.tile_pool(name="hp", bufs=2, space="PSUM") as hp,
        ):
          WT3 = WT.rearrange("(k p) o -> p k o", p=128)  # [128, kt, o]
          AT3 = AT.rearrange("(k p) r -> p k r", p=128)  # [128, kt, r]

          for _rep in range(repeat):
            # resident tensors
            x_sb = xp.tile([128, kt * s], f16)  # x.T, k-chunk-major
            at = cp.tile([128, kt * r], f16)
            baug = cp.tile([r + 1, o], f16)
            haug = cp.tile([r + 1, s], f16)

            def load_w(ob, parts=1):
                w = wp.tile([128, kt * OB], f16, tag="w")
                kq = kt // parts
                for q in range(parts):
                    nc.sync.dma_start(
                        w.rearrange("p (k o) -> p k o", k=kt)[:, q * kq:(q + 1) * kq, :],
                        WT3[:, q * kq:(q + 1) * kq, ob * OB:(ob + 1) * OB],
                    )
                return w

            # small/constant loads first (scalar HWDGE queue) so compute can
            # start with the first x chunk; W streams on the sync queue
            nc.scalar.dma_start(baug[:, :], Baug[:, :])
            nc.scalar.dma_start(at.rearrange("p (k r) -> p k r", k=kt), AT3[:, :, :])
            # row r stays 1.0 (bias row); rows 0:r are overwritten by the h phase
            nc.vector.memset(haug[:, :], 1.0)

            # x chunks alternate between the two HWDGE queues; the first W
            # chunk is interleaved in quarters among the early x chunks so
            # both h-phase and main matmuls can start almost immediately
            w_cur = None

            def _load_w0_part():
                nonlocal w_cur
                if w_cur is None:
                    w_cur = wp.tile([128, kt * OB], f16, tag="w")
                q = _load_w0_part.q
                kq = kt // 4
                nc.sync.dma_start(
                    w_cur.rearrange("p (k o) -> p k o", k=kt)[:, q * kq:(q + 1) * kq, :],
                    WT3[:, q * kq:(q + 1) * kq, 0:OB],
                )
                _load_w0_part.q += 1

            _load_w0_part.q = 0
            for k in range(kt):
                eng = nc.scalar if k % 2 == 0 else nc.sync
                eng.dma_start(x_sb[:, k * s:(k + 1) * s], xT[k * 128:(k + 1) * 128, :])
                if k < 4:
                    _load_w0_part()

            # h phase: haug[0:r, :] = (x @ A.T).T, fp16
            for sb in range(nsb):
                ht = hp.tile([r, sbk], f32)
                for k in range(kt):
                    nc.tensor.matmul(
                        ht[:],
                        at[:, k * r:(k + 1) * r],
                        x_sb[:, k * s + sb * sbk: k * s + (sb + 1) * sbk],
                        start=(k == 0),
                        stop=(k == kt - 1),
                    )
                nc.vector.tensor_copy(haug[0:r, sb * sbk:(sb + 1) * sbk], ht[:])

            # main: out[mt*128:+128, ob*OB:+OB] = sum_k xT_k.T @ WT_k + haug.T @ baug
            for ob in range(nob):
                w = w_cur
                if ob + 1 < nob:
                    w_cur = load_w(ob + 1)
                else:
                    w_cur = None
                for mt in range(mt_n):
                    pt = pp.tile([128, OB], f32)
                    for k in range(kt):
                        nc.tensor.matmul(
                            pt[:],
                            x_sb[:, k * s + mt * 128: k * s + mt * 128 + 128],
                            w[:, k * OB:(k + 1) * OB],
                            start=(k == 0),
                            stop=False,
                        )
                    nc.tensor.matmul(
                        pt[:],
                        haug[:, mt * 128: mt * 128 + 128],
                        baug[:, ob * OB:(ob + 1) * OB],
                        start=False,
                        stop=True,
                    )
                    ot = op.tile([128, OB], f32)
                    nc.vector.tensor_copy(ot[:], pt[:])
                    nc.scalar.dma_start(
                        out[mt * 128:(mt + 1) * 128, ob * OB:(ob + 1) * OB], ot[:]
                    )
    nc.compile()
    return nc


def prep_in_maps(data, W, bias, lora_a, lora_b):
    WT16 = np.ascontiguousarray(W.astype(np.float16).T)  # [I, O]
    bias16 = bias.astype(np.float16)
    in_maps = []
    for b in range(data.shape[0]):
        g = b // (data.shape[0] // G)
        in_maps.append({
            "xT": np.ascontiguousarray(data[b].astype(np.float16).T),  # [I, S]
            "WT": WT16,
            "AT": np.ascontiguousarray(lora_a[g].astype(np.float16).T),  # [I, R]
            "Baug": np.concatenate(
                [(2.0 * lora_b[g].T).astype(np.float16), bias16[None, :]], axis=0
            ),  # [R+1, O]
        })
    return in_maps


def kernel(data, W, bias, lora_a, lora_b):
    from concourse.bass_utils import run_bass_kernel_spmd

    if "nc" not in _CACHE:
        _CACHE["nc"] = build()
    nc = _CACHE["nc"]
    in_maps = prep_in_maps(data, W, bias, lora_a, lora_b)
    res = run_bass_kernel_spmd(nc, in_maps, list(range(len(in_maps))))
    return np.stack([res.results[c]["out"] for c in range(len(in_maps))], axis=0)
